# revision 6
# baseline (speedup 1.0000x reference)
"""Trainium2 Bass kernel for 4-D valid convolution.

Problem: inputs [2, 64, 18, 18, 18, 18] fp32, kernel [81, 64, 64] fp32
(81 = 3^4 offsets row-major over (dw, dx, dy, dz)), output
[2, 64, 16, 16, 16, 16] fp32.

Sharding (8 cores): batch (2) x output-W chunks (4 chunks of 4).  Each core
receives input slabs x[b, :, w0:w0+6] plus the full kernel, and produces
out[b, :, w0:w0+4] as [64, 4, 16, 16, 16].

The PE moving-operand fetch is bandwidth-limited (~1 TB/s measured on
strided APs), so the layout is built for contiguity: the z dim is
pre-sliced into three shifted copies C_d = x[..., d:d+16] (d = 0,1,2),
stored [X, Y, 16z] so every matmul moving stream reads 512-byte
contiguous, 32B-aligned runs (16y x 16z fp16).  Column index of (X,Y,z)
is X*288 + Y*16 + z.

Two SBUF dup-tiles per slab (128 partitions each):
  T : rows 0-63 = C1, rows 64-127 = C0.  A K=128 matmul at column q
      covers (dz=0, dz=1) of a (dw,dx,dy) triple -> 27 "A pairs".
  T2: rows 0-63 = C2 shifted +16 cols (one y-row), rows 64-127 = C2.
      K=128 matmul covers (dy=0,dz=2)+(dy=1,dz=2) -> 9 "B pairs";
      the 9 leftover singles (dy=2,dz=2) run K=64 4-way packed: tile A
      on PE quadrant (0,0) reading T2 lo rows, tile B on (64,64)
      reading T2 hi rows - the two stream concurrently.
Two output tiles (x0..x0+1, x0+2..x0+3) run in PE col-groups 0/64.
81 stream-slots per tile-pair at ~256B/col.  PSUM: pairs accumulate in
banks pa/pb, singles in pc/pd, reduced with ACT copy + lane-aligned DVE
adds, then DMA'd out.  DMA loads are issued in need-order round-robin
across the sync/scalar/gpsimd rings so the matmul gate data lands first.
"""

import numpy as np

B, CIN, COUT = 2, 64, 64
S = 18          # input spatial per dim
SO = 16         # output spatial per dim
NW = 4          # output w per core
NSLAB = 6       # input w slabs per core
XPL = S * SO              # 288 cols per x-plane
TC_ = S * XPL             # 5184 cols of T per slab
T2C = TC_ + 16            # T2 cols incl the +16 (one y-row) shift

_CACHE = {}


def _build_nc(dt_in):
    import concourse.bass as bass
    import concourse.mybir as mybir

    f32 = mybir.dt.float32

    nc = bass.Bass()
    xt_h = nc.dram_tensor("xt", [128, NSLAB, TC_], dt_in, kind="ExternalInput")
    x2_h = nc.dram_tensor("x2", [128, NSLAB, T2C], dt_in, kind="ExternalInput")
    # weights pre-arranged on host to match moving-row pairing:
    # wa: rows 0-63 = kernel[(dw,dx,dy), dz=1], rows 64-127 = dz=0
    # wb: rows 0-63 = kernel[(dw,dx), dy=0, dz=2], rows 64-127 = dy=1
    # ws: both halves = kernel[(dw,dx), dy=2, dz=2]
    wa_h = nc.dram_tensor("wa", [128, 27, COUT], dt_in, kind="ExternalInput")
    wb_h = nc.dram_tensor("wb", [128, 9, COUT], dt_in, kind="ExternalInput")
    ws_h = nc.dram_tensor("ws", [128, 9, COUT], dt_in, kind="ExternalInput")
    out_h = nc.dram_tensor(
        "out", [COUT, NW, SO, SO, SO], f32, kind="ExternalOutput"
    )

    tc = _make_tile_context(nc)
    with tc:
        with (
            tc.tile_pool(name="xp", bufs=1) as xpool,
            tc.tile_pool(name="wp", bufs=1) as wpool,
            tc.tile_pool(name="ob", bufs=3) as opool,
            tc.tile_pool(name="ps", bufs=2, space="PSUM") as ppool,
        ):
            # ---- DMA ring round-robin (shared for loads and stores).
            dma_engines = [nc.sync, nc.scalar, nc.gpsimd]
            dma_rr = [0]

            def dma(dst, src):
                dma_engines[dma_rr[0] % 3].dma_start(dst, src)
                dma_rr[0] += 1

            wa = wpool.tile([128, 27, COUT], dt_in, tag="wa")
            wb = wpool.tile([128, 9, COUT], dt_in, tag="wb")
            ws = wpool.tile([128, 9, COUT], dt_in, tag="ws")
            # +32 slack cols: the rearrange view spans q0+576 > loaded cols
            # at the max window start (reads never touch the slack).
            xts, x2s = [], []
            for s in range(NSLAB):
                xt_t = xpool.tile([128, TC_ + 32], dt_in, tag=f"xt{s}")
                x2_t = xpool.tile([128, T2C + 32], dt_in, tag=f"x2{s}")
                xts.append(xt_t)
                x2s.append(x2_t)

            HT = TC_ // 2
            H2 = T2C // 2

            def load_t(s):
                dma(xts[s][:, 0:HT], xt_h[:, s, 0:HT])
                dma(xts[s][:, HT:TC_], xt_h[:, s, HT:TC_])

            def load_2(s):
                dma(x2s[s][:, 0:H2], x2_h[:, s, 0:H2])
                dma(x2s[s][:, H2:T2C], x2_h[:, s, H2:T2C])

            # need-ordered load schedule: matmul gate data first.
            dma(wa[:], wa_h[:])
            load_t(0)
            load_t(1)
            load_t(2)
            dma(wb[:], wb_h[:])
            load_2(0)
            load_2(1)
            load_2(2)
            dma(ws[:], ws_h[:])
            for s in range(3, NSLAB):
                load_t(s)
                load_2(s)

            def rhs(xt, prange, q0):
                # [p, 2x, 16y, 16z] view: 512B-contiguous runs per x-plane
                v = xt[prange, q0 : q0 + 576]
                v = v.rearrange("p (x y z) -> p x y z", x=2, y=S, z=16)
                return v[:, :, 0:16, :]

            PFULL = slice(0, 128)
            PLO = slice(0, 64)
            PHI = slice(64, 128)

            # ---- main loop: 16 tile-pairs ----
            for w in range(NW):
                for x0 in (0, 4, 8, 12):
                    pa = ppool.tile([128, 512], f32, tag="pA")
                    pb = ppool.tile([128, 512], f32, tag="pB")
                    pc = ppool.tile([128, 512], f32, tag="pC")
                    pd = ppool.tile([128, 512], f32, tag="pD")

                    # A phase: (dz=0,1) pairs, 27 K=128 matmuls per tile
                    for j in range(27):
                        dw, dx, dy = j // 9, (j // 3) % 3, j % 3
                        xt = xts[w + dw]
                        qa = (x0 + dx) * XPL + dy * 16
                        st = j == 0
                        nc.tensor.matmul(
                            pa[0:64, :],
                            wa[:, j, :],
                            rhs(xt, PFULL, qa),
                            start=st, stop=False,
                            tile_position=(0, 0),
                        )
                        nc.tensor.matmul(
                            pb[64:128, :],
                            wa[:, j, :],
                            rhs(xt, PFULL, qa + 576),
                            start=st, stop=False,
                            tile_position=(0, 64),
                        )

                    # B phase: (dy=0,1 @ dz=2) pairs, 9 K=128 matmuls/tile.
                    # hi rows C2[q] pair with dy=1 -> q = dx*288 + 1*16
                    for j2 in range(9):
                        dw, dx = j2 // 3, j2 % 3
                        x2 = x2s[w + dw]
                        qb = (x0 + dx) * XPL + 16
                        last = j2 == 8
                        nc.tensor.matmul(
                            pa[0:64, :],
                            wb[:, j2, :],
                            rhs(x2, PFULL, qb),
                            start=False, stop=last,
                            tile_position=(0, 0),
                        )
                        nc.tensor.matmul(
                            pb[64:128, :],
                            wb[:, j2, :],
                            rhs(x2, PFULL, qb + 576),
                            start=False, stop=last,
                            tile_position=(0, 64),
                        )

                    # singles: (dy=2, dz=2), K=64, 4-way packed: tile A on
                    # quadrant (0,0) reads T2 lo rows (C2 shifted +16, so
                    # q+48 -> C2[q+32]); tile B on (64,64) reads T2 hi rows
                    # (C2[q+32] directly).  The two stream concurrently.
                    for j2 in range(9):
                        dw, dx = j2 // 3, j2 % 3
                        x2 = x2s[w + dw]
                        st = j2 == 0
                        last = j2 == 8
                        qs_lo = (x0 + dx) * XPL + 48
                        qs_hi = (x0 + 2 + dx) * XPL + 32
                        nc.tensor.matmul(
                            pc[0:64, :],
                            ws[0:64, j2, :],
                            rhs(x2, PLO, qs_lo),
                            start=st, stop=last,
                            tile_position=(0, 0),
                        )
                        nc.tensor.matmul(
                            pd[64:128, :],
                            ws[64:128, j2, :],
                            rhs(x2, PHI, qs_hi),
                            start=st, stop=last,
                            tile_position=(64, 64),
                        )

                    # epilogue: bank adds (lane-aligned) + store.  DVE cannot
                    # read two PSUM operands in one op: ACT copies C/D to
                    # SBUF, DVE adds A/B (single PSUM read) into it.
                    osb = opool.tile([128, 512], f32, tag="osb")
                    nc.scalar.copy(osb[0:64, :], pc[0:64, :])
                    nc.scalar.copy(osb[64:128, :], pd[64:128, :])
                    nc.vector.tensor_add(
                        out=osb[0:64, :], in0=pa[0:64, :], in1=osb[0:64, :]
                    )
                    nc.vector.tensor_add(
                        out=osb[64:128, :], in0=pb[64:128, :], in1=osb[64:128, :]
                    )
                    lo = osb[0:64, :].rearrange(
                        "p (x y z) -> p x y z", x=2, y=16, z=16
                    )
                    hi = osb[64:128, :].rearrange(
                        "p (x y z) -> p x y z", x=2, y=16, z=16
                    )
                    dma(out_h[:, w, x0 : x0 + 2, :, :], lo)
                    dma(out_h[:, w, x0 + 2 : x0 + 4, :, :], hi)

    _split_multiwaits(nc)
    return nc


def _make_tile_context(nc):
    from concourse.tile import TileContext

    class TC(TileContext):
        # stock teardown is drain -> barrier -> sem-clear -> barrier; the
        # final barrier only orders engine-stream ends and costs ~2us.
        def _drain_and_barrier(self, tick_clock, wait_clock):
            from concourse.vector_clock import ScopedClock

            nc = self.nc
            drain_inst = nc.sync.drain()
            wait_clock.add_sem_waits(
                drain_inst.ins, ScopedClock({None: tick_clock.global_clock})
            )
            nc.all_engine_barrier()
            assert self.sems is not None
            popped = nc._tile_sem_poison_stack.pop()
            assert popped is self._sem_poison
            nc.clear_and_free_semaphores(list(self.sems.allocated().values()))

    return TC(nc)


def _split_multiwaits(nc, max_waits=1):
    """The walrus build here rejects any instruction carrying more than one
    sync-wait ("Too many sync wait commands").  Tile attaches one wait per
    outstanding producer.  Move excess waits onto same-engine NoOps inserted
    immediately before the instruction - semantically identical."""
    import concourse.mybir as mybir

    n_split = 0
    for fn in nc.m.functions:
        for blk in fn.blocks:
            out = []
            for inst in list(blk.instructions):
                si = inst.sync_info
                if si is not None and si.on_wait and len(si.on_wait) > max_waits:
                    waits = list(si.on_wait)
                    extra = waits[:-max_waits]
                    for k in range(0, len(extra), max_waits):
                        nop = mybir.InstNoOp(
                            name=f"{inst.name}.w{k}", ins=[], outs=[]
                        )
                        nop.engine = inst.engine
                        nop.sync_info = mybir.SyncInfo(
                            on_wait=extra[k : k + max_waits], on_update=[]
                        )
                        nc.register_instruction(nop)
                        out.append(nop)
                        n_split += 1
                    si.on_wait = waits[-max_waits:]
                out.append(inst)
            blk.instructions = out
    return n_split


# compute dtype: "float16" (fastest, rel err ~3e-4) or "float32r"
DTYPE = "float16"


def _get_nc():
    if "nc" not in _CACHE:
        import concourse.mybir as mybir

        _CACHE["nc"] = _build_nc(getattr(mybir.dt, DTYPE))
    return _CACHE["nc"]


def _np_dtype():
    if DTYPE == "float16":
        return np.float16
    return np.float32


def _shard_inputs(inputs):
    nd = _np_dtype()
    x = np.asarray(inputs["inputs"], dtype=np.float32).astype(nd)
    wk = np.asarray(inputs["kernel"], dtype=np.float32).astype(nd)
    k5 = wk.reshape(3, 3, 3, 3, CIN, COUT)  # [dw, dx, dy, dz, ci, co]
    k3 = wk.reshape(27, 3, CIN, COUT)
    # wa[(dw,dx,dy)]: lo rows dz=1, hi rows dz=0
    wa = np.ascontiguousarray(
        np.concatenate(
            [k3[:, 1].transpose(1, 0, 2), k3[:, 0].transpose(1, 0, 2)], axis=0
        )
    )
    # wb[(dw,dx)]: lo rows (dy=0,dz=2), hi rows (dy=1,dz=2)
    kb = k5[:, :, :, 2].reshape(9, 3, CIN, COUT)  # [(dw,dx), dy, ci, co]
    wb = np.ascontiguousarray(
        np.concatenate(
            [kb[:, 0].transpose(1, 0, 2), kb[:, 1].transpose(1, 0, 2)], axis=0
        )
    )
    # ws[(dw,dx)]: both halves (dy=2,dz=2)
    wsh = kb[:, 2].transpose(1, 0, 2)
    wsf = np.ascontiguousarray(np.concatenate([wsh, wsh], axis=0))
    in_maps = []
    for c in range(8):
        b, wc = c // 4, c % 4
        w0 = 4 * wc
        sl = x[b, :, w0 : w0 + 6]                     # [CIN, 6, 18, 18, 18]
        c0 = sl[..., 0:16].reshape(CIN, NSLAB, TC_)    # z+0
        c1 = sl[..., 1:17].reshape(CIN, NSLAB, TC_)    # z+1
        c2 = sl[..., 2:18].reshape(CIN, NSLAB, TC_)    # z+2
        xt = np.empty((128, NSLAB, TC_), dtype=nd)
        xt[0:CIN] = c1
        xt[CIN:] = c0
        x2 = np.zeros((128, NSLAB, T2C), dtype=nd)
        x2[0:CIN, :, 16:T2C] = c2                      # lo rows: C2 shifted
        x2[CIN:, :, 0:TC_] = c2                        # hi rows: C2
        in_maps.append({"xt": xt, "x2": x2, "wa": wa, "wb": wb, "ws": wsf})
    return in_maps


def _gather_outputs(results):
    out = np.empty((B, COUT, NW * 4, SO, SO, SO), dtype=np.float32)
    for c in range(8):
        b, wc = c // 4, c % 4
        w0 = 4 * wc
        out[b, :, w0 : w0 + 4] = results[c]["out"]
    return out


def kernel(**inputs):
    from concourse.bass_utils import run_bass_kernel_spmd

    res = run_bass_kernel_spmd(_get_nc(), _shard_inputs(inputs), list(range(8)))
    return _gather_outputs(res.results)


# revision 9
# speedup vs baseline: 1.1470x; 1.1470x over previous
"""Trainium2 Bass kernel for 4-D valid convolution (Winograd F(2,3) in z).

Problem: inputs [2, 64, 18, 18, 18, 18] fp32, kernel [81, 64, 64] fp32
(81 = 3^4 offsets row-major over (dw, dx, dy, dz)), output
[2, 64, 16, 16, 16, 16] fp32.

Sharding (8 cores): batch (2) x output-W chunks (4 chunks of 4).  Each core
gets input slabs x[b, :, w0:w0+6] plus the full kernel, and produces
out[b, :, w0:w0+4] as [64, 4, 16, 16, 16].

The PE moving-operand fetch is byte-bandwidth-limited (~450 B/cycle
aggregate across quadrant streams, measured), so runtime scales with
contraction-bytes x output-positions.  Winograd F(2,3) along z cuts that
by 1/3: the host transforms the input into 4 m-copies per z-tile of 2
(m0 = x0-x2, m1 = x1+x2, m2 = x2-x1, m3 = x1-x3) and the weights into
Gw_k (Gw0 = w[dz=0], Gw1 = (w0+w1+w2)/2, Gw2 = (w0-w1+w2)/2, Gw3 =
w[dz=2]); the PE contracts only over (dw,dx,dy) x cin = 27*64 per
m-point, and the epilogue applies the 2-tap inverse (out_even =
M0+M1+M2, out_odd = M1-M2-M3) with the ACT/DVE bank-combine it already
needed.

Layout per (k, slab): m_k[ci, X(18), Y(18), zt(8)], col = X*144 + Y*16/2
... = X*144 + Y*8 + zt.  SBUF dup-tile D_k: rows 64-127 = m_k, rows
0-63 = m_k shifted +8 cols (one y-row), so a K=128 matmul at column q
covers (dy, dy+1) pairs; dy=2 singles run K=64 4-way packed (tile A on
quadrant (0,0) reading lo rows, tile B on (64,64) reading hi rows,
streaming concurrently).

Tiles: out x-range of 4 planes per tile, two tiles (x0, x0+4) in PE
col-groups 0/64 -> 8 tile-pairs (4 w x 2 x-halves), N=512 streams
(4x*16y*8zt), 4 k-accumulations in two 2-bank PSUM tiles per tile-pair.
Loads are column-pieced (x-planes 0-9 / 10-17) and issued in need order
round-robin over the sync/scalar/gpsimd DMA rings; tile-pair loop is
x-half-major so each half only needs its piece of every slab.
"""

import numpy as np

B, CIN, COUT = 2, 64, 64
S = 18          # input spatial per dim
SO = 16         # output spatial per dim
NW = 4          # output w per core
NSLAB = 6      # input w slabs per core
ZT = 8          # z-tiles (of 2) per output
XPL = S * ZT              # 144 cols per x-plane
DC = S * XPL              # 2592 m-cols per slab
DKC = DC + 8              # D_k dram cols incl the +8 (one y-row) shift
DSL = DKC + 16            # D_k sbuf cols incl rearrange-view slack
P01 = 10 * XPL            # piece split: x-planes 0-9 | 10-17

_CACHE = {}


def _build_nc(dt_in):
    import concourse.bass as bass
    import concourse.mybir as mybir

    f32 = mybir.dt.float32

    nc = bass.Bass()
    d_h = [
        nc.dram_tensor(f"d{k}", [128, NSLAB, DKC], dt_in, kind="ExternalInput")
        for k in range(4)
    ]
    # wp{k}: pair weights, lo rows = Gw_k[(dw,dx), dy=0], hi rows = dy=1
    # ws{k}: single weights, both halves = Gw_k[(dw,dx), dy=2]
    wp_h = [
        nc.dram_tensor(f"wp{k}", [128, 9, COUT], dt_in, kind="ExternalInput")
        for k in range(4)
    ]
    ws_h = [
        nc.dram_tensor(f"ws{k}", [128, 9, COUT], dt_in, kind="ExternalInput")
        for k in range(4)
    ]
    out_h = nc.dram_tensor(
        "out", [COUT, NW, SO, SO, SO], f32, kind="ExternalOutput"
    )

    tc = _make_tile_context(nc)
    with tc:
        with (
            tc.tile_pool(name="xp", bufs=1) as xpool,
            tc.tile_pool(name="wpl", bufs=1) as wpool,
            tc.tile_pool(name="ob", bufs=3) as opool,
            tc.tile_pool(name="ps", bufs=2, space="PSUM") as ppool,
        ):
            dma_engines = [nc.sync, nc.scalar, nc.gpsimd]
            dma_rr = [0]

            def dma(dst, src):
                dma_engines[dma_rr[0] % 3].dma_start(dst, src)
                dma_rr[0] += 1

            wps, wss = [], []
            for k in range(4):
                wp_t = wpool.tile([128, 9, COUT], dt_in, tag=f"wp{k}")
                ws_t = wpool.tile([128, 9, COUT], dt_in, tag=f"ws{k}")
                wps.append(wp_t)
                wss.append(ws_t)
            ds = []
            for k in range(4):
                row = []
                for s in range(NSLAB):
                    d_t = xpool.tile([128, DSL], dt_in, tag=f"d{k}s{s}")
                    row.append(d_t)
                ds.append(row)

            # need-ordered loads: the first tile-pair consumes k-phases in
            # order, each reading slabs 0-2 of its k, so interleave
            # [weights_k, D_k slabs 0-2] per k, then the w>0 slabs, then
            # the second x-half pieces.
            for k in range(4):
                dma(wps[k][:], wp_h[k][:])
                dma(wss[k][:], ws_h[k][:])
                for s in range(3):
                    dma(ds[k][s][:, 0:P01], d_h[k][:, s, 0:P01])
            for s in range(3, NSLAB):
                for k in range(4):
                    dma(ds[k][s][:, 0:P01], d_h[k][:, s, 0:P01])
            for s in range(NSLAB):
                for k in range(4):
                    dma(ds[k][s][:, P01:DKC], d_h[k][:, s, P01:DKC])

            def rhs(t, prange, q0):
                v = t[prange, q0 : q0 + 576]
                v = v.rearrange("p (x y z) -> p x y z", x=4, y=S, z=ZT)
                return v[:, :, 0:16, :]

            PFULL = slice(0, 128)
            PLO = slice(0, 64)
            PHI = slice(64, 128)

            # ---- main loop: 8 tile-pairs, x-half-major ----
            for xh in range(2):
                x0 = 8 * xh       # tile A covers x-planes x0..x0+3
                for w in range(NW):
                    p01 = ppool.tile([128, 2, 512], f32, tag="p01")
                    p23 = ppool.tile([128, 2, 512], f32, tag="p23")
                    pk = [p01[:, 0], p01[:, 1], p23[:, 0], p23[:, 1]]

                    for k in range(4):
                        # dy (0,1) pairs: 9 K=128 matmuls per tile
                        for j2 in range(9):
                            dw, dx = j2 // 3, j2 % 3
                            dt_ = ds[k][w + dw]
                            q = (x0 + dx) * XPL + 8
                            st = j2 == 0
                            nc.tensor.matmul(
                                pk[k][0:64, :],
                                wps[k][:, j2, :],
                                rhs(dt_, PFULL, q),
                                start=st, stop=False,
                                tile_position=(0, 0),
                            )
                            nc.tensor.matmul(
                                pk[k][64:128, :],
                                wps[k][:, j2, :],
                                rhs(dt_, PFULL, q + 4 * XPL),
                                start=st, stop=False,
                                tile_position=(0, 64),
                            )
                        # dy=2 singles: K=64, 4-way packed (A lo / B hi)
                        for j2 in range(9):
                            dw, dx = j2 // 3, j2 % 3
                            dt_ = ds[k][w + dw]
                            last = j2 == 8
                            nc.tensor.matmul(
                                pk[k][0:64, :],
                                wss[k][0:64, j2, :],
                                rhs(dt_, PLO, (x0 + dx) * XPL + 24),
                                start=False, stop=last,
                                tile_position=(0, 0),
                            )
                            nc.tensor.matmul(
                                pk[k][64:128, :],
                                wss[k][64:128, j2, :],
                                rhs(dt_, PHI, (x0 + 4 + dx) * XPL + 16),
                                start=False, stop=last,
                                tile_position=(64, 64),
                            )

                    # epilogue: Winograd inverse along z (even = M0+M1+M2,
                    # odd = M1-M2-M3) via ACT copy (psum->sbuf) + DVE adds
                    # (single PSUM operand per op), then store.
                    osb = opool.tile([128, 512, 2], f32, tag="osb")
                    ev = osb[:, :, 0]
                    od = osb[:, :, 1]
                    import concourse.mybir as mybir

                    nc.scalar.copy(ev, p01[:, 0])
                    nc.vector.tensor_add(out=ev, in0=p01[:, 1], in1=ev)
                    nc.vector.tensor_add(out=ev, in0=p23[:, 0], in1=ev)
                    nc.scalar.copy(od, p01[:, 1])
                    # od = od - M2, od = od - M3 (one PSUM operand per op)
                    nc.vector.scalar_tensor_tensor(
                        out=od, in0=p23[:, 0], scalar=-1.0, in1=od,
                        op0=mybir.AluOpType.mult, op1=mybir.AluOpType.add,
                    )
                    nc.vector.scalar_tensor_tensor(
                        out=od, in0=p23[:, 1], scalar=-1.0, in1=od,
                        op0=mybir.AluOpType.mult, op1=mybir.AluOpType.add,
                    )
                    lo = osb[0:64].rearrange(
                        "p (x y zt) r -> p x y (zt r)", x=4, y=16, zt=ZT
                    )
                    hi = osb[64:128].rearrange(
                        "p (x y zt) r -> p x y (zt r)", x=4, y=16, zt=ZT
                    )
                    dma(out_h[:, w, x0 : x0 + 4, :, :], lo)
                    dma(out_h[:, w, x0 + 4 : x0 + 8, :, :], hi)

    _split_multiwaits(nc)
    return nc


def _make_tile_context(nc):
    from concourse.tile import TileContext

    class TC(TileContext):
        # stock teardown is drain -> barrier -> sem-clear -> barrier; the
        # final barrier only orders engine-stream ends and costs ~2us.
        def _drain_and_barrier(self, tick_clock, wait_clock):
            from concourse.vector_clock import ScopedClock

            nc = self.nc
            drain_inst = nc.sync.drain()
            wait_clock.add_sem_waits(
                drain_inst.ins, ScopedClock({None: tick_clock.global_clock})
            )
            nc.all_engine_barrier()
            assert self.sems is not None
            popped = nc._tile_sem_poison_stack.pop()
            assert popped is self._sem_poison
            nc.clear_and_free_semaphores(list(self.sems.allocated().values()))

    return TC(nc)


def _split_multiwaits(nc, max_waits=1):
    """The walrus build here rejects any instruction carrying more than one
    sync-wait ("Too many sync wait commands").  Tile attaches one wait per
    outstanding producer.  Move excess waits onto same-engine NoOps inserted
    immediately before the instruction - semantically identical."""
    import concourse.mybir as mybir

    n_split = 0
    for fn in nc.m.functions:
        for blk in fn.blocks:
            out = []
            for inst in list(blk.instructions):
                si = inst.sync_info
                if si is not None and si.on_wait and len(si.on_wait) > max_waits:
                    waits = list(si.on_wait)
                    extra = waits[:-max_waits]
                    for k in range(0, len(extra), max_waits):
                        nop = mybir.InstNoOp(
                            name=f"{inst.name}.w{k}", ins=[], outs=[]
                        )
                        nop.engine = inst.engine
                        nop.sync_info = mybir.SyncInfo(
                            on_wait=extra[k : k + max_waits], on_update=[]
                        )
                        nc.register_instruction(nop)
                        out.append(nop)
                        n_split += 1
                    si.on_wait = waits[-max_waits:]
                out.append(inst)
            blk.instructions = out
    return n_split


# compute dtype: "float16" (fastest, rel err ~4e-4) or "float32r"
DTYPE = "float16"


def _get_nc():
    if "nc" not in _CACHE:
        import concourse.mybir as mybir

        _CACHE["nc"] = _build_nc(getattr(mybir.dt, DTYPE))
    return _CACHE["nc"]


def _np_dtype():
    if DTYPE == "float16":
        return np.float16
    return np.float32


def _shard_inputs(inputs):
    nd = _np_dtype()
    x = np.asarray(inputs["inputs"], dtype=np.float32)
    wk = np.asarray(inputs["kernel"], dtype=np.float32)
    k5 = wk.reshape(3, 3, 3, 3, CIN, COUT)  # [dw, dx, dy, dz, ci, co]
    # weight transform Gw_k over dz
    w0, w1, w2 = k5[:, :, :, 0], k5[:, :, :, 1], k5[:, :, :, 2]
    gw = [w0, (w0 + w1 + w2) * 0.5, (w0 - w1 + w2) * 0.5, w2]
    wps, wss = [], []
    for k in range(4):
        g = gw[k].reshape(9, 3, CIN, COUT)  # [(dw,dx), dy, ci, co]
        wp = np.concatenate(
            [g[:, 0].transpose(1, 0, 2), g[:, 1].transpose(1, 0, 2)], axis=0
        )
        w2h = g[:, 2].transpose(1, 0, 2)
        ws_ = np.concatenate([w2h, w2h], axis=0)
        wps.append(np.ascontiguousarray(wp.astype(nd)))
        wss.append(np.ascontiguousarray(ws_.astype(nd)))
    in_maps = []
    for c in range(8):
        b, wc = c // 4, c % 4
        w0c = 4 * wc
        sl = x[b, :, w0c : w0c + 6]             # [CIN, 6, 18, 18, 18] fp32
        ze = sl[..., 0::2]                      # z even: 0,2,..,16 (9)
        zo = sl[..., 1::2]                      # z odd: 1,3,..,17 (9)
        # m_k[ci, s, X, Y, zt], zt = 0..7: windows z = 2zt .. 2zt+3
        m = [
            ze[..., 0:8] - ze[..., 1:9],        # x0 - x2
            zo[..., 0:8] + ze[..., 1:9],        # x1 + x2
            ze[..., 1:9] - zo[..., 0:8],        # x2 - x1
            zo[..., 0:8] - zo[..., 1:9],        # x1 - x3
        ]
        feeds = {}
        for k in range(4):
            mk = m[k].reshape(CIN, NSLAB, DC).astype(nd)
            dk = np.zeros((128, NSLAB, DKC), dtype=nd)
            dk[0:CIN, :, 8:DKC] = mk            # lo rows: m_k[c-8]
            dk[CIN:, :, 0:DC] = mk              # hi rows: m_k[c]
            feeds[f"d{k}"] = dk
            feeds[f"wp{k}"] = wps[k]
            feeds[f"ws{k}"] = wss[k]
        in_maps.append(feeds)
    return in_maps


def _gather_outputs(results):
    out = np.empty((B, COUT, NW * 4, SO, SO, SO), dtype=np.float32)
    for c in range(8):
        b, wc = c // 4, c % 4
        w0 = 4 * wc
        out[b, :, w0 : w0 + 4] = results[c]["out"]
    return out


def kernel(**inputs):
    from concourse.bass_utils import run_bass_kernel_spmd

    res = run_bass_kernel_spmd(_get_nc(), _shard_inputs(inputs), list(range(8)))
    return _gather_outputs(res.results)


# revision 10
# speedup vs baseline: 1.1888x; 1.0365x over previous
"""Trainium2 Bass kernel for 4-D valid convolution (Winograd F(2,3) in z).

Problem: inputs [2, 64, 18, 18, 18, 18] fp32, kernel [81, 64, 64] fp32
(81 = 3^4 offsets row-major over (dw, dx, dy, dz)), output
[2, 64, 16, 16, 16, 16] fp32.

Sharding (8 cores): batch (2) x output-W chunks (4 chunks of 4).  Each core
gets input slabs x[b, :, w0:w0+6] plus the full kernel, and produces
out[b, :, w0:w0+4] as [64, 4, 16, 16, 16].

The PE moving-operand fetch is byte-bandwidth-limited (~450 B/cycle
aggregate across quadrant streams, measured), so runtime scales with
contraction-bytes x output-positions.  Winograd F(2,3) along z cuts that
by 1/3: the host transforms the input into 4 m-copies per z-tile of 2
(m0 = x0-x2, m1 = x1+x2, m2 = x2-x1, m3 = x1-x3) and the weights into
Gw_k (Gw0 = w[dz=0], Gw1 = (w0+w1+w2)/2, Gw2 = (w0-w1+w2)/2, Gw3 =
w[dz=2]); the PE contracts only over (dw,dx,dy) x cin = 27*64 per
m-point, and the epilogue applies the 2-tap inverse (out_even =
M0+M1+M2, out_odd = M1-M2-M3) with the ACT/DVE bank-combine it already
needed.

Layout per (k, slab): m_k[ci, X(18), Y(18), zt(8)], col = X*144 + Y*16/2
... = X*144 + Y*8 + zt.  SBUF dup-tile D_k: rows 64-127 = m_k, rows
0-63 = m_k shifted +8 cols (one y-row), so a K=128 matmul at column q
covers (dy, dy+1) pairs; dy=2 singles run K=64 4-way packed (tile A on
quadrant (0,0) reading lo rows, tile B on (64,64) reading hi rows,
streaming concurrently).

Tiles: out x-range of 4 planes per tile, two tiles (x0, x0+4) in PE
col-groups 0/64 -> 8 tile-pairs (4 w x 2 x-halves), N=512 streams
(4x*16y*8zt), 4 k-accumulations in two 2-bank PSUM tiles per tile-pair.
Loads are column-pieced (x-planes 0-9 / 10-17) and issued in need order
round-robin over the sync/scalar/gpsimd DMA rings; tile-pair loop is
x-half-major so each half only needs its piece of every slab.
"""

import numpy as np

B, CIN, COUT = 2, 64, 64
S = 18          # input spatial per dim
SO = 16         # output spatial per dim
NW = 4          # output w per core
NSLAB = 6      # input w slabs per core
ZT = 8          # z-tiles (of 2) per output
XPL = S * ZT              # 144 cols per x-plane
DC = S * XPL              # 2592 m-cols per slab
DKC = DC + 8              # D_k dram cols incl the +8 (one y-row) shift
DSL = DKC + 16            # D_k sbuf cols incl rearrange-view slack
P01 = 10 * XPL            # piece split: x-planes 0-9 | 10-17

_CACHE = {}


def _build_nc(dt_in):
    import concourse.bass as bass
    import concourse.mybir as mybir

    f32 = mybir.dt.float32

    nc = bass.Bass()
    d_h = [
        nc.dram_tensor(f"d{k}", [128, NSLAB, DKC], dt_in, kind="ExternalInput")
        for k in range(4)
    ]
    # wp{k}: pair weights, lo rows = Gw_k[(dw,dx), dy=0], hi rows = dy=1
    # ws{k}: single weights, both halves = Gw_k[(dw,dx), dy=2]
    wp_h = [
        nc.dram_tensor(f"wp{k}", [128, 9, COUT], dt_in, kind="ExternalInput")
        for k in range(4)
    ]
    ws_h = [
        nc.dram_tensor(f"ws{k}", [128, 9, COUT], dt_in, kind="ExternalInput")
        for k in range(4)
    ]
    out_h = nc.dram_tensor(
        "out", [COUT, NW, SO, SO, SO], f32, kind="ExternalOutput"
    )

    tc = _make_tile_context(nc)
    with tc:
        with (
            tc.tile_pool(name="xp", bufs=1) as xpool,
            tc.tile_pool(name="wpl", bufs=1) as wpool,
            tc.tile_pool(name="ob", bufs=3) as opool,
            tc.tile_pool(name="ps", bufs=2, space="PSUM") as ppool,
        ):
            # scalar (ACT) issues no DMA: trigger instructions stall on
            # DMA-ring backpressure in the engine FIFO, which would delay
            # the epilogue ACTIVATEs behind them (and the PSUM bank frees
            # the next tile-pair waits on).
            dma_engines = [nc.sync, nc.gpsimd]
            dma_rr = [0]

            def dma(dst, src):
                dma_engines[dma_rr[0] % 2].dma_start(dst, src)
                dma_rr[0] += 1

            wps, wss = [], []
            for k in range(4):
                wp_t = wpool.tile([128, 9, COUT], dt_in, tag=f"wp{k}")
                ws_t = wpool.tile([128, 9, COUT], dt_in, tag=f"ws{k}")
                wps.append(wp_t)
                wss.append(ws_t)
            ds = []
            for k in range(4):
                row = []
                for s in range(NSLAB):
                    d_t = xpool.tile([128, DSL], dt_in, tag=f"d{k}s{s}")
                    row.append(d_t)
                ds.append(row)

            # need-ordered loads: the first tile-pair consumes k-phases in
            # order, each reading slabs 0-2 of its k, so interleave
            # [weights_k, D_k slabs 0-2] per k, then the w>0 slabs, then
            # the second x-half pieces.
            for k in range(4):
                dma(wps[k][:], wp_h[k][:])
                dma(wss[k][:], ws_h[k][:])
                for s in range(3):
                    dma(ds[k][s][:, 0:P01], d_h[k][:, s, 0:P01])
            for s in range(3, NSLAB):
                for k in range(4):
                    dma(ds[k][s][:, 0:P01], d_h[k][:, s, 0:P01])
            for s in range(NSLAB):
                for k in range(4):
                    dma(ds[k][s][:, P01:DKC], d_h[k][:, s, P01:DKC])

            def rhs(t, prange, q0):
                v = t[prange, q0 : q0 + 576]
                v = v.rearrange("p (x y z) -> p x y z", x=4, y=S, z=ZT)
                return v[:, :, 0:16, :]

            PFULL = slice(0, 128)
            PLO = slice(0, 64)
            PHI = slice(64, 128)

            # ---- main loop: 8 tile-pairs, x-half-major ----
            for xh in range(2):
                x0 = 8 * xh       # tile A covers x-planes x0..x0+3
                for w in range(NW):
                    p01 = ppool.tile([128, 2, 512], f32, tag="p01")
                    p23 = ppool.tile([128, 2, 512], f32, tag="p23")
                    pk = [p01[:, 0], p01[:, 1], p23[:, 0], p23[:, 1]]

                    for k in range(4):
                        # dy (0,1) pairs: 9 K=128 matmuls per tile
                        for j2 in range(9):
                            dw, dx = j2 // 3, j2 % 3
                            dt_ = ds[k][w + dw]
                            q = (x0 + dx) * XPL + 8
                            st = j2 == 0
                            nc.tensor.matmul(
                                pk[k][0:64, :],
                                wps[k][:, j2, :],
                                rhs(dt_, PFULL, q),
                                start=st, stop=False,
                                tile_position=(0, 0),
                            )
                            nc.tensor.matmul(
                                pk[k][64:128, :],
                                wps[k][:, j2, :],
                                rhs(dt_, PFULL, q + 4 * XPL),
                                start=st, stop=False,
                                tile_position=(0, 64),
                            )
                        # dy=2 singles: K=64, 4-way packed (A lo / B hi)
                        for j2 in range(9):
                            dw, dx = j2 // 3, j2 % 3
                            dt_ = ds[k][w + dw]
                            last = j2 == 8
                            nc.tensor.matmul(
                                pk[k][0:64, :],
                                wss[k][0:64, j2, :],
                                rhs(dt_, PLO, (x0 + dx) * XPL + 24),
                                start=False, stop=last,
                                tile_position=(0, 0),
                            )
                            nc.tensor.matmul(
                                pk[k][64:128, :],
                                wss[k][64:128, j2, :],
                                rhs(dt_, PHI, (x0 + 4 + dx) * XPL + 16),
                                start=False, stop=last,
                                tile_position=(64, 64),
                            )

                    # epilogue: Winograd inverse along z (even = M0+M1+M2,
                    # odd = M1-M2-M3) via ACT copy (psum->sbuf) + DVE adds
                    # (single PSUM operand per op), then store.
                    osb = opool.tile([128, 512, 2], f32, tag="osb")
                    ev = osb[:, :, 0]
                    od = osb[:, :, 1]
                    import concourse.mybir as mybir

                    nc.scalar.copy(ev, p01[:, 0])
                    nc.vector.tensor_add(out=ev, in0=p01[:, 1], in1=ev)
                    nc.vector.tensor_add(out=ev, in0=p23[:, 0], in1=ev)
                    nc.scalar.copy(od, p01[:, 1])
                    # od = od - M2, od = od - M3 (one PSUM operand per op)
                    nc.vector.scalar_tensor_tensor(
                        out=od, in0=p23[:, 0], scalar=-1.0, in1=od,
                        op0=mybir.AluOpType.mult, op1=mybir.AluOpType.add,
                    )
                    nc.vector.scalar_tensor_tensor(
                        out=od, in0=p23[:, 1], scalar=-1.0, in1=od,
                        op0=mybir.AluOpType.mult, op1=mybir.AluOpType.add,
                    )
                    lo = osb[0:64].rearrange(
                        "p (x y zt) r -> p x y (zt r)", x=4, y=16, zt=ZT
                    )
                    hi = osb[64:128].rearrange(
                        "p (x y zt) r -> p x y (zt r)", x=4, y=16, zt=ZT
                    )
                    dma(out_h[:, w, x0 : x0 + 4, :, :], lo)
                    dma(out_h[:, w, x0 + 4 : x0 + 8, :, :], hi)

    _split_multiwaits(nc)
    return nc


def _make_tile_context(nc):
    from concourse.tile import TileContext

    class TC(TileContext):
        # stock teardown is drain -> barrier -> sem-clear -> barrier; the
        # final barrier only orders engine-stream ends and costs ~2us.
        def _drain_and_barrier(self, tick_clock, wait_clock):
            from concourse.vector_clock import ScopedClock

            nc = self.nc
            drain_inst = nc.sync.drain()
            wait_clock.add_sem_waits(
                drain_inst.ins, ScopedClock({None: tick_clock.global_clock})
            )
            nc.all_engine_barrier()
            assert self.sems is not None
            popped = nc._tile_sem_poison_stack.pop()
            assert popped is self._sem_poison
            nc.clear_and_free_semaphores(list(self.sems.allocated().values()))

    return TC(nc)


def _split_multiwaits(nc, max_waits=1):
    """The walrus build here rejects any instruction carrying more than one
    sync-wait ("Too many sync wait commands").  Tile attaches one wait per
    outstanding producer.  Move excess waits onto same-engine NoOps inserted
    immediately before the instruction - semantically identical."""
    import concourse.mybir as mybir

    n_split = 0
    for fn in nc.m.functions:
        for blk in fn.blocks:
            out = []
            for inst in list(blk.instructions):
                si = inst.sync_info
                if si is not None and si.on_wait and len(si.on_wait) > max_waits:
                    waits = list(si.on_wait)
                    extra = waits[:-max_waits]
                    for k in range(0, len(extra), max_waits):
                        nop = mybir.InstNoOp(
                            name=f"{inst.name}.w{k}", ins=[], outs=[]
                        )
                        nop.engine = inst.engine
                        nop.sync_info = mybir.SyncInfo(
                            on_wait=extra[k : k + max_waits], on_update=[]
                        )
                        nc.register_instruction(nop)
                        out.append(nop)
                        n_split += 1
                    si.on_wait = waits[-max_waits:]
                out.append(inst)
            blk.instructions = out
    return n_split


# compute dtype: "float16" (fastest, rel err ~4e-4) or "float32r"
DTYPE = "float16"


def _get_nc():
    if "nc" not in _CACHE:
        import concourse.mybir as mybir

        _CACHE["nc"] = _build_nc(getattr(mybir.dt, DTYPE))
    return _CACHE["nc"]


def _np_dtype():
    if DTYPE == "float16":
        return np.float16
    return np.float32


def _shard_inputs(inputs):
    nd = _np_dtype()
    x = np.asarray(inputs["inputs"], dtype=np.float32)
    wk = np.asarray(inputs["kernel"], dtype=np.float32)
    k5 = wk.reshape(3, 3, 3, 3, CIN, COUT)  # [dw, dx, dy, dz, ci, co]
    # weight transform Gw_k over dz
    w0, w1, w2 = k5[:, :, :, 0], k5[:, :, :, 1], k5[:, :, :, 2]
    gw = [w0, (w0 + w1 + w2) * 0.5, (w0 - w1 + w2) * 0.5, w2]
    wps, wss = [], []
    for k in range(4):
        g = gw[k].reshape(9, 3, CIN, COUT)  # [(dw,dx), dy, ci, co]
        wp = np.concatenate(
            [g[:, 0].transpose(1, 0, 2), g[:, 1].transpose(1, 0, 2)], axis=0
        )
        w2h = g[:, 2].transpose(1, 0, 2)
        ws_ = np.concatenate([w2h, w2h], axis=0)
        wps.append(np.ascontiguousarray(wp.astype(nd)))
        wss.append(np.ascontiguousarray(ws_.astype(nd)))
    in_maps = []
    for c in range(8):
        b, wc = c // 4, c % 4
        w0c = 4 * wc
        sl = x[b, :, w0c : w0c + 6]             # [CIN, 6, 18, 18, 18] fp32
        ze = sl[..., 0::2]                      # z even: 0,2,..,16 (9)
        zo = sl[..., 1::2]                      # z odd: 1,3,..,17 (9)
        # m_k[ci, s, X, Y, zt], zt = 0..7: windows z = 2zt .. 2zt+3
        m = [
            ze[..., 0:8] - ze[..., 1:9],        # x0 - x2
            zo[..., 0:8] + ze[..., 1:9],        # x1 + x2
            ze[..., 1:9] - zo[..., 0:8],        # x2 - x1
            zo[..., 0:8] - zo[..., 1:9],        # x1 - x3
        ]
        feeds = {}
        for k in range(4):
            mk = m[k].reshape(CIN, NSLAB, DC).astype(nd)
            dk = np.zeros((128, NSLAB, DKC), dtype=nd)
            dk[0:CIN, :, 8:DKC] = mk            # lo rows: m_k[c-8]
            dk[CIN:, :, 0:DC] = mk              # hi rows: m_k[c]
            feeds[f"d{k}"] = dk
            feeds[f"wp{k}"] = wps[k]
            feeds[f"ws{k}"] = wss[k]
        in_maps.append(feeds)
    return in_maps


def _gather_outputs(results):
    out = np.empty((B, COUT, NW * 4, SO, SO, SO), dtype=np.float32)
    for c in range(8):
        b, wc = c // 4, c % 4
        w0 = 4 * wc
        out[b, :, w0 : w0 + 4] = results[c]["out"]
    return out


def kernel(**inputs):
    from concourse.bass_utils import run_bass_kernel_spmd

    res = run_bass_kernel_spmd(_get_nc(), _shard_inputs(inputs), list(range(8)))
    return _gather_outputs(res.results)


# revision 13
# speedup vs baseline: 1.1954x; 1.0056x over previous
"""Trainium2 Bass kernel for 4-D valid convolution (Winograd F(2,3) in z).

Problem: inputs [2, 64, 18, 18, 18, 18] fp32, kernel [81, 64, 64] fp32
(81 = 3^4 offsets row-major over (dw, dx, dy, dz)), output
[2, 64, 16, 16, 16, 16] fp32.

Sharding (8 cores): batch (2) x output-W chunks (4 chunks of 4).  Each core
gets input slabs x[b, :, w0:w0+6] plus the full kernel, and produces
out[b, :, w0:w0+4] as [64, 4, 16, 16, 16].

The PE moving-operand fetch is byte-bandwidth-limited (~450 B/cycle
aggregate across quadrant streams, measured), so runtime scales with
contraction-bytes x output-positions.  Winograd F(2,3) along z cuts that
by 1/3: the host transforms the input into 4 m-copies per z-tile of 2
(m0 = x0-x2, m1 = x1+x2, m2 = x2-x1, m3 = x1-x3) and the weights into
Gw_k (Gw0 = w[dz=0], Gw1 = (w0+w1+w2)/2, Gw2 = (w0-w1+w2)/2, Gw3 =
w[dz=2]); the PE contracts only over (dw,dx,dy) x cin = 27*64 per
m-point, and the epilogue applies the 2-tap inverse (out_even =
M0+M1+M2, out_odd = M1-M2-M3) with the ACT/DVE bank-combine it already
needed.

Layout per (k, slab): m_k[ci, X(18), Y(18), zt(8)], col = X*144 + Y*16/2
... = X*144 + Y*8 + zt.  SBUF dup-tile D_k: rows 64-127 = m_k, rows
0-63 = m_k shifted +8 cols (one y-row), so a K=128 matmul at column q
covers (dy, dy+1) pairs; dy=2 singles run K=64 4-way packed (tile A on
quadrant (0,0) reading lo rows, tile B on (64,64) reading hi rows,
streaming concurrently).

Tiles: out x-range of 4 planes per tile, two tiles (x0, x0+4) in PE
col-groups 0/64 -> 8 tile-pairs (4 w x 2 x-halves), N=512 streams
(4x*16y*8zt), 4 k-accumulations in two 2-bank PSUM tiles per tile-pair.
Loads are column-pieced (x-planes 0-9 / 10-17) and issued in need order
round-robin over the sync/scalar/gpsimd DMA rings; tile-pair loop is
x-half-major so each half only needs its piece of every slab.
"""

import numpy as np

B, CIN, COUT = 2, 64, 64
S = 18          # input spatial per dim
SO = 16         # output spatial per dim
NW = 4          # output w per core
NSLAB = 6      # input w slabs per core
ZT = 8          # z-tiles (of 2) per output
XPL = S * ZT              # 144 cols per x-plane
DC = S * XPL              # 2592 m-cols per slab
DKC = DC + 8              # D_k dram cols incl the +8 (one y-row) shift
DSL = DKC + 16            # D_k sbuf cols incl rearrange-view slack
P01 = 10 * XPL            # piece split: x-planes 0-9 | 10-17

_CACHE = {}


def _build_nc(dt_in):
    import concourse.bass as bass
    import concourse.mybir as mybir

    f32 = mybir.dt.float32

    nc = bass.Bass()
    d_h = [
        nc.dram_tensor(f"d{k}", [128, NSLAB, DKC], dt_in, kind="ExternalInput")
        for k in range(4)
    ]
    # wp{k}: pair weights, lo rows = Gw_k[(dw,dx), dy=0], hi rows = dy=1
    # ws{k}: single weights, both halves = Gw_k[(dw,dx), dy=2]
    wp_h = [
        nc.dram_tensor(f"wp{k}", [128, 9, COUT], dt_in, kind="ExternalInput")
        for k in range(4)
    ]
    ws_h = [
        nc.dram_tensor(f"ws{k}", [128, 9, COUT], dt_in, kind="ExternalInput")
        for k in range(4)
    ]
    out_h = nc.dram_tensor(
        "out", [COUT, NW, SO, SO, SO], f32, kind="ExternalOutput"
    )

    tc = _make_tile_context(nc)
    with tc:
        with (
            tc.tile_pool(name="xp", bufs=1) as xpool,
            tc.tile_pool(name="wpl", bufs=1) as wpool,
            tc.tile_pool(name="ob", bufs=3) as opool,
            tc.tile_pool(name="ps", bufs=2, space="PSUM") as ppool,
        ):
            # scalar (ACT) issues no DMA: trigger instructions stall on
            # DMA-ring backpressure in the engine FIFO, which would delay
            # the epilogue ACTIVATEs behind them (and the PSUM bank frees
            # the next tile-pair waits on).
            dma_engines = [nc.sync, nc.gpsimd]
            dma_rr = [0]

            def dma(dst, src):
                dma_engines[dma_rr[0] % 2].dma_start(dst, src)
                dma_rr[0] += 1

            wps, wss = [], []
            for k in range(4):
                wp_t = wpool.tile([128, 9, COUT], dt_in, tag=f"wp{k}")
                ws_t = wpool.tile([128, 9, COUT], dt_in, tag=f"ws{k}")
                wps.append(wp_t)
                wss.append(ws_t)
            ds = []
            for k in range(4):
                row = []
                for s in range(NSLAB):
                    d_t = xpool.tile([128, DSL], dt_in, tag=f"d{k}s{s}")
                    row.append(d_t)
                ds.append(row)

            # need-ordered loads: the first tile-pair consumes k-phases in
            # order, each reading slabs 0-2 of its k, so interleave
            # [weights_k, D_k slabs 0-2] per k, then the w>0 slabs, then
            # the second x-half pieces.  The k=0 gate data is split in
            # column halves so both rings transfer it in parallel.
            def dma2(dst, src, n):
                h = n // 2
                dma(dst[:, 0:h], src[:, 0:h])
                dma(dst[:, h:n], src[:, h:n])

            for k in range(4):
                if k == 0:
                    dma(wps[0][:], wp_h[0][:])
                    dma(wss[0][:], ws_h[0][:])
                    for s in range(3):
                        dma2(ds[0][s], d_h[0][:, s], P01)
                else:
                    dma(wps[k][:], wp_h[k][:])
                    dma(wss[k][:], ws_h[k][:])
                    for s in range(3):
                        dma(ds[k][s][:, 0:P01], d_h[k][:, s, 0:P01])
            for s in range(3, NSLAB):
                for k in range(4):
                    dma(ds[k][s][:, 0:P01], d_h[k][:, s, 0:P01])
            for s in range(NSLAB):
                for k in range(4):
                    dma(ds[k][s][:, P01:DKC], d_h[k][:, s, P01:DKC])

            def rhs(t, prange, q0):
                v = t[prange, q0 : q0 + 576]
                v = v.rearrange("p (x y z) -> p x y z", x=4, y=S, z=ZT)
                return v[:, :, 0:16, :]

            PFULL = slice(0, 128)
            PLO = slice(0, 64)
            PHI = slice(64, 128)

            # ---- main loop: 8 tile-pairs, x-half-major ----
            for xh in range(2):
                x0 = 8 * xh       # tile A covers x-planes x0..x0+3
                for w in range(NW):
                    p01 = ppool.tile([128, 2, 512], f32, tag="p01")
                    p23 = ppool.tile([128, 2, 512], f32, tag="p23")
                    pk = [p01[:, 0], p01[:, 1], p23[:, 0], p23[:, 1]]

                    for k in range(4):
                        # dy (0,1) pairs: 9 K=128 matmuls per tile
                        for j2 in range(9):
                            dw, dx = j2 // 3, j2 % 3
                            dt_ = ds[k][w + dw]
                            q = (x0 + dx) * XPL + 8
                            st = j2 == 0
                            nc.tensor.matmul(
                                pk[k][0:64, :],
                                wps[k][:, j2, :],
                                rhs(dt_, PFULL, q),
                                start=st, stop=False,
                                tile_position=(0, 0),
                            )
                            nc.tensor.matmul(
                                pk[k][64:128, :],
                                wps[k][:, j2, :],
                                rhs(dt_, PFULL, q + 4 * XPL),
                                start=st, stop=False,
                                tile_position=(0, 64),
                            )
                        # dy=2 singles: K=64, 4-way packed (A lo / B hi)
                        for j2 in range(9):
                            dw, dx = j2 // 3, j2 % 3
                            dt_ = ds[k][w + dw]
                            last = j2 == 8
                            nc.tensor.matmul(
                                pk[k][0:64, :],
                                wss[k][0:64, j2, :],
                                rhs(dt_, PLO, (x0 + dx) * XPL + 24),
                                start=False, stop=last,
                                tile_position=(0, 0),
                            )
                            nc.tensor.matmul(
                                pk[k][64:128, :],
                                wss[k][64:128, j2, :],
                                rhs(dt_, PHI, (x0 + 4 + dx) * XPL + 16),
                                start=False, stop=last,
                                tile_position=(64, 64),
                            )

                    # epilogue: Winograd inverse along z (even = M0+M1+M2,
                    # odd = M1-M2-M3) via ACT copy (psum->sbuf) + DVE adds
                    # (single PSUM operand per op), then store.
                    osb = opool.tile([128, 512, 2], f32, tag="osb")
                    ev = osb[:, :, 0]
                    od = osb[:, :, 1]
                    import concourse.mybir as mybir

                    # column-halved so ACT and DVE pipeline (halves the
                    # exposed epilogue latency and PSUM-free delay)
                    for h0 in (0, 256):
                        hs = slice(h0, h0 + 256)
                        evh, odh = ev[:, hs], od[:, hs]
                        nc.scalar.copy(evh, p01[:, 0, hs])
                        nc.vector.tensor_add(
                            out=evh, in0=p01[:, 1, hs], in1=evh
                        )
                        nc.vector.tensor_add(
                            out=evh, in0=p23[:, 0, hs], in1=evh
                        )
                        nc.scalar.copy(odh, p01[:, 1, hs])
                        # odh -= M2, odh -= M3 (one PSUM operand per op)
                        nc.vector.scalar_tensor_tensor(
                            out=odh, in0=p23[:, 0, hs], scalar=-1.0, in1=odh,
                            op0=mybir.AluOpType.mult, op1=mybir.AluOpType.add,
                        )
                        nc.vector.scalar_tensor_tensor(
                            out=odh, in0=p23[:, 1, hs], scalar=-1.0, in1=odh,
                            op0=mybir.AluOpType.mult, op1=mybir.AluOpType.add,
                        )
                    lo = osb[0:64].rearrange(
                        "p (x y zt) r -> p x y (zt r)", x=4, y=16, zt=ZT
                    )
                    hi = osb[64:128].rearrange(
                        "p (x y zt) r -> p x y (zt r)", x=4, y=16, zt=ZT
                    )
                    # x-split stores land on both rings in parallel
                    dma(out_h[:, w, x0 : x0 + 2, :, :], lo[:, 0:2])
                    dma(out_h[:, w, x0 + 2 : x0 + 4, :, :], lo[:, 2:4])
                    dma(out_h[:, w, x0 + 4 : x0 + 6, :, :], hi[:, 0:2])
                    dma(out_h[:, w, x0 + 6 : x0 + 8, :, :], hi[:, 2:4])

    _split_multiwaits(nc)
    return nc


def _make_tile_context(nc):
    from concourse.tile import TileContext

    class TC(TileContext):
        # stock teardown is drain -> barrier -> sem-clear -> barrier; the
        # final barrier only orders engine-stream ends and costs ~2us.
        def _drain_and_barrier(self, tick_clock, wait_clock):
            from concourse.vector_clock import ScopedClock

            nc = self.nc
            drain_inst = nc.sync.drain()
            wait_clock.add_sem_waits(
                drain_inst.ins, ScopedClock({None: tick_clock.global_clock})
            )
            nc.all_engine_barrier()
            assert self.sems is not None
            popped = nc._tile_sem_poison_stack.pop()
            assert popped is self._sem_poison
            nc.clear_and_free_semaphores(list(self.sems.allocated().values()))

    return TC(nc)


def _split_multiwaits(nc, max_waits=1):
    """The walrus build here rejects any instruction carrying more than one
    sync-wait ("Too many sync wait commands").  Tile attaches one wait per
    outstanding producer.  Move excess waits onto same-engine NoOps inserted
    immediately before the instruction - semantically identical."""
    import concourse.mybir as mybir

    n_split = 0
    for fn in nc.m.functions:
        for blk in fn.blocks:
            out = []
            for inst in list(blk.instructions):
                si = inst.sync_info
                if si is not None and si.on_wait and len(si.on_wait) > max_waits:
                    waits = list(si.on_wait)
                    extra = waits[:-max_waits]
                    for k in range(0, len(extra), max_waits):
                        nop = mybir.InstNoOp(
                            name=f"{inst.name}.w{k}", ins=[], outs=[]
                        )
                        nop.engine = inst.engine
                        nop.sync_info = mybir.SyncInfo(
                            on_wait=extra[k : k + max_waits], on_update=[]
                        )
                        nc.register_instruction(nop)
                        out.append(nop)
                        n_split += 1
                    si.on_wait = waits[-max_waits:]
                out.append(inst)
            blk.instructions = out
    return n_split


# compute dtype: "float16" (fastest, rel err ~4e-4) or "float32r"
DTYPE = "float16"


def _get_nc():
    if "nc" not in _CACHE:
        import concourse.mybir as mybir

        _CACHE["nc"] = _build_nc(getattr(mybir.dt, DTYPE))
    return _CACHE["nc"]


def _np_dtype():
    if DTYPE == "float16":
        return np.float16
    return np.float32


def _shard_inputs(inputs):
    nd = _np_dtype()
    x = np.asarray(inputs["inputs"], dtype=np.float32)
    wk = np.asarray(inputs["kernel"], dtype=np.float32)
    k5 = wk.reshape(3, 3, 3, 3, CIN, COUT)  # [dw, dx, dy, dz, ci, co]
    # weight transform Gw_k over dz
    w0, w1, w2 = k5[:, :, :, 0], k5[:, :, :, 1], k5[:, :, :, 2]
    gw = [w0, (w0 + w1 + w2) * 0.5, (w0 - w1 + w2) * 0.5, w2]
    wps, wss = [], []
    for k in range(4):
        g = gw[k].reshape(9, 3, CIN, COUT)  # [(dw,dx), dy, ci, co]
        wp = np.concatenate(
            [g[:, 0].transpose(1, 0, 2), g[:, 1].transpose(1, 0, 2)], axis=0
        )
        w2h = g[:, 2].transpose(1, 0, 2)
        ws_ = np.concatenate([w2h, w2h], axis=0)
        wps.append(np.ascontiguousarray(wp.astype(nd)))
        wss.append(np.ascontiguousarray(ws_.astype(nd)))
    in_maps = []
    for c in range(8):
        b, wc = c // 4, c % 4
        w0c = 4 * wc
        sl = x[b, :, w0c : w0c + 6]             # [CIN, 6, 18, 18, 18] fp32
        ze = sl[..., 0::2]                      # z even: 0,2,..,16 (9)
        zo = sl[..., 1::2]                      # z odd: 1,3,..,17 (9)
        # m_k[ci, s, X, Y, zt], zt = 0..7: windows z = 2zt .. 2zt+3
        m = [
            ze[..., 0:8] - ze[..., 1:9],        # x0 - x2
            zo[..., 0:8] + ze[..., 1:9],        # x1 + x2
            ze[..., 1:9] - zo[..., 0:8],        # x2 - x1
            zo[..., 0:8] - zo[..., 1:9],        # x1 - x3
        ]
        feeds = {}
        for k in range(4):
            mk = m[k].reshape(CIN, NSLAB, DC).astype(nd)
            dk = np.zeros((128, NSLAB, DKC), dtype=nd)
            dk[0:CIN, :, 8:DKC] = mk            # lo rows: m_k[c-8]
            dk[CIN:, :, 0:DC] = mk              # hi rows: m_k[c]
            feeds[f"d{k}"] = dk
            feeds[f"wp{k}"] = wps[k]
            feeds[f"ws{k}"] = wss[k]
        in_maps.append(feeds)
    return in_maps


def _gather_outputs(results):
    out = np.empty((B, COUT, NW * 4, SO, SO, SO), dtype=np.float32)
    for c in range(8):
        b, wc = c // 4, c % 4
        w0 = 4 * wc
        out[b, :, w0 : w0 + 4] = results[c]["out"]
    return out


def kernel(**inputs):
    from concourse.bass_utils import run_bass_kernel_spmd

    res = run_bass_kernel_spmd(_get_nc(), _shard_inputs(inputs), list(range(8)))
    return _gather_outputs(res.results)


# revision 18
# speedup vs baseline: 1.2195x; 1.0202x over previous
"""Trainium2 Bass kernel for 4-D valid convolution (Winograd F(2,3) in z).

Problem: inputs [2, 64, 18, 18, 18, 18] fp32, kernel [81, 64, 64] fp32
(81 = 3^4 offsets row-major over (dw, dx, dy, dz)), output
[2, 64, 16, 16, 16, 16] fp32.

Sharding (8 cores): batch (2) x output-W chunks (4 chunks of 4).  Each core
gets input slabs x[b, :, w0:w0+6] plus the full kernel, and produces
out[b, :, w0:w0+4] as [64, 4, 16, 16, 16].

The PE moving-operand fetch is byte-bandwidth-limited (~450 B/cycle
aggregate across quadrant streams, measured), so runtime scales with
contraction-bytes x output-positions.  Winograd F(2,3) along z cuts that
by 1/3: the host transforms the input into 4 m-copies per z-tile of 2
(m0 = x0-x2, m1 = x1+x2, m2 = x2-x1, m3 = x1-x3) and the weights into
Gw_k (Gw0 = w[dz=0], Gw1 = (w0+w1+w2)/2, Gw2 = (w0-w1+w2)/2, Gw3 =
w[dz=2]); the PE contracts only over (dw,dx,dy) x cin = 27*64 per
m-point, and the epilogue applies the 2-tap inverse (out_even =
M0+M1+M2, out_odd = M1-M2-M3) with the ACT/DVE bank-combine it already
needed.

Layout per (k, slab): m_k[ci, X(18), Y(18), zt(8)], col = X*144 + Y*16/2
... = X*144 + Y*8 + zt.  SBUF dup-tile D_k: rows 64-127 = m_k, rows
0-63 = m_k shifted +8 cols (one y-row), so a K=128 matmul at column q
covers (dy, dy+1) pairs; dy=2 singles run K=64 4-way packed (tile A on
quadrant (0,0) reading lo rows, tile B on (64,64) reading hi rows,
streaming concurrently).

Tiles: out x-range of 4 planes per tile, two tiles (x0, x0+4) in PE
col-groups 0/64 -> 8 tile-pairs (4 w x 2 x-halves), N=512 streams
(4x*16y*8zt), 4 k-accumulations in two 2-bank PSUM tiles per tile-pair.
Loads are column-pieced (x-planes 0-9 / 10-17) and issued in need order
round-robin over the sync/scalar/gpsimd DMA rings; tile-pair loop is
x-half-major so each half only needs its piece of every slab.
"""

import numpy as np

B, CIN, COUT = 2, 64, 64
S = 18          # input spatial per dim
SO = 16         # output spatial per dim
NW = 4          # output w per core
NSLAB = 6      # input w slabs per core
ZT = 8          # z-tiles (of 2) per output
XPL = S * ZT              # 144 cols per x-plane
DC = S * XPL              # 2592 m-cols per slab
DKC = DC + 8              # D_k dram cols incl the +8 (one y-row) shift
DSL = DKC + 16            # D_k sbuf cols incl rearrange-view slack
P01 = 10 * XPL            # piece split: x-planes 0-9 | 10-17

_CACHE = {}


def _build_nc(dt_in):
    import concourse.bass as bass
    import concourse.mybir as mybir

    f32 = mybir.dt.float32

    nc = bass.Bass()
    d_h = [
        nc.dram_tensor(f"d{k}", [128, NSLAB, DKC], dt_in, kind="ExternalInput")
        for k in range(4)
    ]
    # wp{k}: pair weights, lo rows = Gw_k[(dw,dx), dy=0], hi rows = dy=1
    # ws{k}: single weights, both halves = Gw_k[(dw,dx), dy=2]
    wp_h = [
        nc.dram_tensor(f"wp{k}", [128, 9, COUT], dt_in, kind="ExternalInput")
        for k in range(4)
    ]
    ws_h = [
        nc.dram_tensor(f"ws{k}", [128, 9, COUT], dt_in, kind="ExternalInput")
        for k in range(4)
    ]
    out_h = nc.dram_tensor(
        "out", [COUT, NW, SO, SO, SO], f32, kind="ExternalOutput"
    )

    tc = _make_tile_context(nc)
    with tc:
        with (
            tc.tile_pool(name="xp", bufs=1) as xpool,
            tc.tile_pool(name="wpl", bufs=1) as wpool,
            tc.tile_pool(name="ob", bufs=3) as opool,
            tc.tile_pool(name="ps", bufs=2, space="PSUM") as ppool,
        ):
            # scalar (ACT) issues no DMA: trigger instructions stall on
            # DMA-ring backpressure in the engine FIFO, which would delay
            # the epilogue ACTIVATEs behind them (and the PSUM bank frees
            # the next tile-pair waits on).
            dma_engines = [nc.sync, nc.gpsimd]
            dma_rr = [0]

            def dma(dst, src):
                dma_engines[dma_rr[0] % 2].dma_start(dst, src)
                dma_rr[0] += 1

            wps, wss = [], []
            for k in range(4):
                wp_t = wpool.tile([128, 9, COUT], dt_in, tag=f"wp{k}")
                ws_t = wpool.tile([128, 9, COUT], dt_in, tag=f"ws{k}")
                wps.append(wp_t)
                wss.append(ws_t)
            ds = []
            for k in range(4):
                row = []
                for s in range(NSLAB):
                    d_t = xpool.tile([128, DSL], dt_in, tag=f"d{k}s{s}")
                    row.append(d_t)
                ds.append(row)

            # need-ordered loads: the first tile-pair consumes k-phases in
            # order, each reading slabs 0-2 of its k, so interleave
            # [weights_k, D_k slabs 0-2] per k, then the w>0 slabs, then
            # the second x-half pieces.  The k=0 gate data is split in
            # column halves so both rings transfer it in parallel.
            def dma2(dst, src, n):
                h = n // 2
                dma(dst[:, 0:h], src[:, 0:h])
                dma(dst[:, h:n], src[:, h:n])

            for k in range(4):
                if k == 0:
                    dma(wps[0][:], wp_h[0][:])
                    dma(wss[0][:], ws_h[0][:])
                    for s in range(3):
                        dma2(ds[0][s], d_h[0][:, s], P01)
                else:
                    dma(wps[k][:], wp_h[k][:])
                    dma(wss[k][:], ws_h[k][:])
                    for s in range(3):
                        dma(ds[k][s][:, 0:P01], d_h[k][:, s, 0:P01])
            for s in range(3, NSLAB):
                for k in range(4):
                    dma(ds[k][s][:, 0:P01], d_h[k][:, s, 0:P01])
            for s in range(NSLAB):
                for k in range(4):
                    dma(ds[k][s][:, P01:DKC], d_h[k][:, s, P01:DKC])

            # HAM warmup: the PE clock-gate runs cold (1.2 GHz) until
            # ~3.4us of sustained matmul activity.  Dependency-free
            # matmuls on never-written scratch warm it up during the DMA
            # gate so the real stream starts at 2.4 GHz.  Output goes to
            # the p01-tag rotation slot (overwritten by tile-pair 1).
            warm_ps = ppool.tile([128, 2, 512], f32, tag="p01")
            wscr = xpool.tile([128, 640], dt_in, tag="wscr")
            nc.vector.memset(wscr[:], 0.5)
            for _ in range(10):
                nc.tensor.matmul(
                    warm_ps[:, 0][0:64, :],
                    wscr[:, 0:64],
                    wscr[:, 64:576],
                    start=True, stop=True,
                    tile_position=(0, 0),
                )

            def rhs(t, prange, q0):
                v = t[prange, q0 : q0 + 576]
                v = v.rearrange("p (x y z) -> p x y z", x=4, y=S, z=ZT)
                return v[:, :, 0:16, :]

            PFULL = slice(0, 128)
            PLO = slice(0, 64)
            PHI = slice(64, 128)

            # ---- main loop: 8 tile-pairs, x-half-major ----
            for xh in range(2):
                x0 = 8 * xh       # tile A covers x-planes x0..x0+3
                for w in range(NW):
                    p01 = ppool.tile([128, 2, 512], f32, tag="p01")
                    p23 = ppool.tile([128, 2, 512], f32, tag="p23")
                    pk = [p01[:, 0], p01[:, 1], p23[:, 0], p23[:, 1]]

                    for k in range(4):
                        # dy (0,1) pairs: 9 K=128 matmuls per tile
                        for j2 in range(9):
                            dw, dx = j2 // 3, j2 % 3
                            dt_ = ds[k][w + dw]
                            q = (x0 + dx) * XPL + 8
                            st = j2 == 0
                            nc.tensor.matmul(
                                pk[k][0:64, :],
                                wps[k][:, j2, :],
                                rhs(dt_, PFULL, q),
                                start=st, stop=False,
                                tile_position=(0, 0),
                            )
                            nc.tensor.matmul(
                                pk[k][64:128, :],
                                wps[k][:, j2, :],
                                rhs(dt_, PFULL, q + 4 * XPL),
                                start=st, stop=False,
                                tile_position=(0, 64),
                            )
                        # dy=2 singles: K=64, 4-way packed (A lo / B hi)
                        for j2 in range(9):
                            dw, dx = j2 // 3, j2 % 3
                            dt_ = ds[k][w + dw]
                            last = j2 == 8
                            nc.tensor.matmul(
                                pk[k][0:64, :],
                                wss[k][0:64, j2, :],
                                rhs(dt_, PLO, (x0 + dx) * XPL + 24),
                                start=False, stop=last,
                                tile_position=(0, 0),
                            )
                            nc.tensor.matmul(
                                pk[k][64:128, :],
                                wss[k][64:128, j2, :],
                                rhs(dt_, PHI, (x0 + 4 + dx) * XPL + 16),
                                start=False, stop=last,
                                tile_position=(64, 64),
                            )

                    # epilogue: Winograd inverse along z (even = M0+M1+M2,
                    # odd = M1-M2-M3) via ACT copy (psum->sbuf) + DVE adds
                    # (single PSUM operand per op), then store.
                    osb = opool.tile([128, 512, 2], f32, tag="osb")
                    ev = osb[:, :, 0]
                    od = osb[:, :, 1]
                    import concourse.mybir as mybir

                    # column-halved so ACT and DVE pipeline (halves the
                    # exposed epilogue latency and PSUM-free delay)
                    for h0 in (0, 256):
                        hs = slice(h0, h0 + 256)
                        evh, odh = ev[:, hs], od[:, hs]
                        nc.scalar.copy(evh, p01[:, 0, hs])
                        nc.vector.tensor_add(
                            out=evh, in0=p01[:, 1, hs], in1=evh
                        )
                        nc.vector.tensor_add(
                            out=evh, in0=p23[:, 0, hs], in1=evh
                        )
                        nc.scalar.copy(odh, p01[:, 1, hs])
                        # odh -= M2, odh -= M3 (one PSUM operand per op)
                        nc.vector.scalar_tensor_tensor(
                            out=odh, in0=p23[:, 0, hs], scalar=-1.0, in1=odh,
                            op0=mybir.AluOpType.mult, op1=mybir.AluOpType.add,
                        )
                        nc.vector.scalar_tensor_tensor(
                            out=odh, in0=p23[:, 1, hs], scalar=-1.0, in1=odh,
                            op0=mybir.AluOpType.mult, op1=mybir.AluOpType.add,
                        )
                    lo = osb[0:64].rearrange(
                        "p (x y zt) r -> p x y (zt r)", x=4, y=16, zt=ZT
                    )
                    hi = osb[64:128].rearrange(
                        "p (x y zt) r -> p x y (zt r)", x=4, y=16, zt=ZT
                    )
                    # stores on the HWDGE (sync) ring only: SWDGE stores
                    # would hold up the gpsimd teardown drain ~2us.
                    nc.sync.dma_start(out_h[:, w, x0 : x0 + 2, :, :], lo[:, 0:2])
                    nc.sync.dma_start(out_h[:, w, x0 + 2 : x0 + 4, :, :], lo[:, 2:4])
                    nc.sync.dma_start(out_h[:, w, x0 + 4 : x0 + 6, :, :], hi[:, 0:2])
                    nc.sync.dma_start(out_h[:, w, x0 + 6 : x0 + 8, :, :], hi[:, 2:4])

    _split_multiwaits(nc)
    return nc


def _make_tile_context(nc):
    from concourse.tile import TileContext

    class TC(TileContext):
        # stock teardown is drain -> barrier -> sem-clear -> barrier; the
        # final barrier only orders engine-stream ends and costs ~2us.
        def _drain_and_barrier(self, tick_clock, wait_clock):
            from concourse.vector_clock import ScopedClock

            nc = self.nc
            drain_inst = nc.sync.drain()
            wait_clock.add_sem_waits(
                drain_inst.ins, ScopedClock({None: tick_clock.global_clock})
            )
            nc.all_engine_barrier()
            assert self.sems is not None
            popped = nc._tile_sem_poison_stack.pop()
            assert popped is self._sem_poison
            nc.clear_and_free_semaphores(list(self.sems.allocated().values()))

    return TC(nc)


def _split_multiwaits(nc, max_waits=1):
    """The walrus build here rejects any instruction carrying more than one
    sync-wait ("Too many sync wait commands").  Tile attaches one wait per
    outstanding producer.  Move excess waits onto same-engine NoOps inserted
    immediately before the instruction - semantically identical."""
    import concourse.mybir as mybir

    n_split = 0
    for fn in nc.m.functions:
        for blk in fn.blocks:
            out = []
            for inst in list(blk.instructions):
                si = inst.sync_info
                if si is not None and si.on_wait and len(si.on_wait) > max_waits:
                    waits = list(si.on_wait)
                    extra = waits[:-max_waits]
                    for k in range(0, len(extra), max_waits):
                        nop = mybir.InstNoOp(
                            name=f"{inst.name}.w{k}", ins=[], outs=[]
                        )
                        nop.engine = inst.engine
                        nop.sync_info = mybir.SyncInfo(
                            on_wait=extra[k : k + max_waits], on_update=[]
                        )
                        nc.register_instruction(nop)
                        out.append(nop)
                        n_split += 1
                    si.on_wait = waits[-max_waits:]
                out.append(inst)
            blk.instructions = out
    return n_split


# compute dtype: "float16" (fastest, rel err ~4e-4) or "float32r"
DTYPE = "float16"


def _get_nc():
    if "nc" not in _CACHE:
        import concourse.mybir as mybir

        _CACHE["nc"] = _build_nc(getattr(mybir.dt, DTYPE))
    return _CACHE["nc"]


def _np_dtype():
    if DTYPE == "float16":
        return np.float16
    return np.float32


def _shard_inputs(inputs):
    nd = _np_dtype()
    x = np.asarray(inputs["inputs"], dtype=np.float32)
    wk = np.asarray(inputs["kernel"], dtype=np.float32)
    k5 = wk.reshape(3, 3, 3, 3, CIN, COUT)  # [dw, dx, dy, dz, ci, co]
    # weight transform Gw_k over dz
    w0, w1, w2 = k5[:, :, :, 0], k5[:, :, :, 1], k5[:, :, :, 2]
    gw = [w0, (w0 + w1 + w2) * 0.5, (w0 - w1 + w2) * 0.5, w2]
    wps, wss = [], []
    for k in range(4):
        g = gw[k].reshape(9, 3, CIN, COUT)  # [(dw,dx), dy, ci, co]
        wp = np.concatenate(
            [g[:, 0].transpose(1, 0, 2), g[:, 1].transpose(1, 0, 2)], axis=0
        )
        w2h = g[:, 2].transpose(1, 0, 2)
        ws_ = np.concatenate([w2h, w2h], axis=0)
        wps.append(np.ascontiguousarray(wp.astype(nd)))
        wss.append(np.ascontiguousarray(ws_.astype(nd)))
    in_maps = []
    for c in range(8):
        b, wc = c // 4, c % 4
        w0c = 4 * wc
        sl = x[b, :, w0c : w0c + 6]             # [CIN, 6, 18, 18, 18] fp32
        ze = sl[..., 0::2]                      # z even: 0,2,..,16 (9)
        zo = sl[..., 1::2]                      # z odd: 1,3,..,17 (9)
        # m_k[ci, s, X, Y, zt], zt = 0..7: windows z = 2zt .. 2zt+3
        m = [
            ze[..., 0:8] - ze[..., 1:9],        # x0 - x2
            zo[..., 0:8] + ze[..., 1:9],        # x1 + x2
            ze[..., 1:9] - zo[..., 0:8],        # x2 - x1
            zo[..., 0:8] - zo[..., 1:9],        # x1 - x3
        ]
        feeds = {}
        for k in range(4):
            mk = m[k].reshape(CIN, NSLAB, DC).astype(nd)
            dk = np.zeros((128, NSLAB, DKC), dtype=nd)
            dk[0:CIN, :, 8:DKC] = mk            # lo rows: m_k[c-8]
            dk[CIN:, :, 0:DC] = mk              # hi rows: m_k[c]
            feeds[f"d{k}"] = dk
            feeds[f"wp{k}"] = wps[k]
            feeds[f"ws{k}"] = wss[k]
        in_maps.append(feeds)
    return in_maps


def _gather_outputs(results):
    out = np.empty((B, COUT, NW * 4, SO, SO, SO), dtype=np.float32)
    for c in range(8):
        b, wc = c // 4, c % 4
        w0 = 4 * wc
        out[b, :, w0 : w0 + 4] = results[c]["out"]
    return out


def kernel(**inputs):
    from concourse.bass_utils import run_bass_kernel_spmd

    res = run_bass_kernel_spmd(_get_nc(), _shard_inputs(inputs), list(range(8)))
    return _gather_outputs(res.results)


# revision 19
# speedup vs baseline: 1.2218x; 1.0018x over previous
"""Trainium2 Bass kernel for 4-D valid convolution (Winograd F(2,3) in z).

Problem: inputs [2, 64, 18, 18, 18, 18] fp32, kernel [81, 64, 64] fp32
(81 = 3^4 offsets row-major over (dw, dx, dy, dz)), output
[2, 64, 16, 16, 16, 16] fp32.

Sharding (8 cores): batch (2) x output-W chunks (4 chunks of 4).  Each core
gets input slabs x[b, :, w0:w0+6] plus the full kernel, and produces
out[b, :, w0:w0+4] as [64, 4, 16, 16, 16].

The PE moving-operand fetch is byte-bandwidth-limited (~450 B/cycle
aggregate across quadrant streams, measured), so runtime scales with
contraction-bytes x output-positions.  Winograd F(2,3) along z cuts that
by 1/3: the host transforms the input into 4 m-copies per z-tile of 2
(m0 = x0-x2, m1 = x1+x2, m2 = x2-x1, m3 = x1-x3) and the weights into
Gw_k (Gw0 = w[dz=0], Gw1 = (w0+w1+w2)/2, Gw2 = (w0-w1+w2)/2, Gw3 =
w[dz=2]); the PE contracts only over (dw,dx,dy) x cin = 27*64 per
m-point, and the epilogue applies the 2-tap inverse (out_even =
M0+M1+M2, out_odd = M1-M2-M3) with the ACT/DVE bank-combine it already
needed.

Layout per (k, slab): m_k[ci, X(18), Y(18), zt(8)], col = X*144 + Y*16/2
... = X*144 + Y*8 + zt.  SBUF dup-tile D_k: rows 64-127 = m_k, rows
0-63 = m_k shifted +8 cols (one y-row), so a K=128 matmul at column q
covers (dy, dy+1) pairs; dy=2 singles run K=64 4-way packed (tile A on
quadrant (0,0) reading lo rows, tile B on (64,64) reading hi rows,
streaming concurrently).

Tiles: out x-range of 4 planes per tile, two tiles (x0, x0+4) in PE
col-groups 0/64 -> 8 tile-pairs (4 w x 2 x-halves), N=512 streams
(4x*16y*8zt), 4 k-accumulations in two 2-bank PSUM tiles per tile-pair.
Loads are column-pieced (x-planes 0-9 / 10-17) and issued in need order
round-robin over the sync/scalar/gpsimd DMA rings; tile-pair loop is
x-half-major so each half only needs its piece of every slab.
"""

import os
import sys

import numpy as np

if "/opt/trn_rl_repo" not in sys.path:
    sys.path.insert(0, "/opt/trn_rl_repo")
os.environ.setdefault("JAX_PLATFORMS", "axon,cpu")

B, CIN, COUT = 2, 64, 64
S = 18          # input spatial per dim
SO = 16         # output spatial per dim
NW = 4          # output w per core
NSLAB = 6      # input w slabs per core
ZT = 8          # z-tiles (of 2) per output
XPL = S * ZT              # 144 cols per x-plane
DC = S * XPL              # 2592 m-cols per slab
DKC = DC + 8              # D_k dram cols incl the +8 (one y-row) shift
DSL = DKC + 16            # D_k sbuf cols incl rearrange-view slack
P01 = 10 * XPL            # piece split: x-planes 0-9 | 10-17

_CACHE = {}


def _build_nc(dt_in):
    import concourse.bass as bass
    import concourse.mybir as mybir

    f32 = mybir.dt.float32

    nc = bass.Bass()
    d_h = [
        nc.dram_tensor(f"d{k}", [128, NSLAB, DKC], dt_in, kind="ExternalInput")
        for k in range(4)
    ]
    # wp{k}: pair weights, lo rows = Gw_k[(dw,dx), dy=0], hi rows = dy=1
    # ws{k}: single weights, both halves = Gw_k[(dw,dx), dy=2]
    wp_h = [
        nc.dram_tensor(f"wp{k}", [128, 9, COUT], dt_in, kind="ExternalInput")
        for k in range(4)
    ]
    ws_h = [
        nc.dram_tensor(f"ws{k}", [128, 9, COUT], dt_in, kind="ExternalInput")
        for k in range(4)
    ]
    out_h = nc.dram_tensor(
        "out", [COUT, NW, SO, SO, SO], f32, kind="ExternalOutput"
    )

    tc = _make_tile_context(nc)
    with tc:
        with (
            tc.tile_pool(name="xp", bufs=1) as xpool,
            tc.tile_pool(name="wpl", bufs=1) as wpool,
            tc.tile_pool(name="ob", bufs=3) as opool,
            tc.tile_pool(name="ps", bufs=2, space="PSUM") as ppool,
        ):
            # scalar (ACT) issues no DMA: trigger instructions stall on
            # DMA-ring backpressure in the engine FIFO, which would delay
            # the epilogue ACTIVATEs behind them (and the PSUM bank frees
            # the next tile-pair waits on).
            dma_engines = [nc.sync, nc.gpsimd]
            dma_rr = [0]

            def dma(dst, src):
                dma_engines[dma_rr[0] % 2].dma_start(dst, src)
                dma_rr[0] += 1

            wps, wss = [], []
            for k in range(4):
                wp_t = wpool.tile([128, 9, COUT], dt_in, tag=f"wp{k}")
                ws_t = wpool.tile([128, 9, COUT], dt_in, tag=f"ws{k}")
                wps.append(wp_t)
                wss.append(ws_t)
            ds = []
            for k in range(4):
                row = []
                for s in range(NSLAB):
                    d_t = xpool.tile([128, DSL], dt_in, tag=f"d{k}s{s}")
                    row.append(d_t)
                ds.append(row)

            # need-ordered loads: the first tile-pair consumes k-phases in
            # order, each reading slabs 0-2 of its k, so interleave
            # [weights_k, D_k slabs 0-2] per k, then the w>0 slabs, then
            # the second x-half pieces.  The k=0 gate data is split in
            # column halves so both rings transfer it in parallel.
            def dma2(dst, src, n):
                h = n // 2
                dma(dst[:, 0:h], src[:, 0:h])
                dma(dst[:, h:n], src[:, h:n])

            for k in range(4):
                if k == 0:
                    dma(wps[0][:], wp_h[0][:])
                    dma(wss[0][:], ws_h[0][:])
                    for s in range(3):
                        dma2(ds[0][s], d_h[0][:, s], P01)
                else:
                    dma(wps[k][:], wp_h[k][:])
                    dma(wss[k][:], ws_h[k][:])
                    for s in range(3):
                        dma(ds[k][s][:, 0:P01], d_h[k][:, s, 0:P01])
            for s in range(3, NSLAB):
                for k in range(4):
                    dma(ds[k][s][:, 0:P01], d_h[k][:, s, 0:P01])
            for s in range(NSLAB):
                for k in range(4):
                    dma(ds[k][s][:, P01:DKC], d_h[k][:, s, P01:DKC])

            # HAM warmup: the PE clock-gate runs cold (1.2 GHz) until
            # ~3.4us of sustained matmul activity.  Dependency-free
            # matmuls on never-written scratch warm it up during the DMA
            # gate so the real stream starts at 2.4 GHz.  Output goes to
            # the p01-tag rotation slot (overwritten by tile-pair 1).
            warm_ps = ppool.tile([128, 2, 512], f32, tag="p01")
            wscr = xpool.tile([128, 640], dt_in, tag="wscr")
            nc.vector.memset(wscr[:], 0.5)
            for _ in range(10):
                nc.tensor.matmul(
                    warm_ps[:, 0][0:64, :],
                    wscr[:, 0:64],
                    wscr[:, 64:576],
                    start=True, stop=True,
                    tile_position=(0, 0),
                )

            def rhs(t, prange, q0):
                v = t[prange, q0 : q0 + 576]
                v = v.rearrange("p (x y z) -> p x y z", x=4, y=S, z=ZT)
                return v[:, :, 0:16, :]

            PFULL = slice(0, 128)
            PLO = slice(0, 64)
            PHI = slice(64, 128)

            # ---- main loop: 8 tile-pairs, x-half-major ----
            for xh in range(2):
                x0 = 8 * xh       # tile A covers x-planes x0..x0+3
                for w in range(NW):
                    p01 = ppool.tile([128, 2, 512], f32, tag="p01")
                    p23 = ppool.tile([128, 2, 512], f32, tag="p23")
                    pk = [p01[:, 0], p01[:, 1], p23[:, 0], p23[:, 1]]

                    for k in range(4):
                        # dy (0,1) pairs: 9 K=128 matmuls per tile
                        for j2 in range(9):
                            dw, dx = j2 // 3, j2 % 3
                            dt_ = ds[k][w + dw]
                            q = (x0 + dx) * XPL + 8
                            st = j2 == 0
                            nc.tensor.matmul(
                                pk[k][0:64, :],
                                wps[k][:, j2, :],
                                rhs(dt_, PFULL, q),
                                start=st, stop=False,
                                tile_position=(0, 0),
                            )
                            nc.tensor.matmul(
                                pk[k][64:128, :],
                                wps[k][:, j2, :],
                                rhs(dt_, PFULL, q + 4 * XPL),
                                start=st, stop=False,
                                tile_position=(0, 64),
                            )
                        # dy=2 singles: K=64, 4-way packed (A lo / B hi)
                        for j2 in range(9):
                            dw, dx = j2 // 3, j2 % 3
                            dt_ = ds[k][w + dw]
                            last = j2 == 8
                            nc.tensor.matmul(
                                pk[k][0:64, :],
                                wss[k][0:64, j2, :],
                                rhs(dt_, PLO, (x0 + dx) * XPL + 24),
                                start=False, stop=last,
                                tile_position=(0, 0),
                            )
                            nc.tensor.matmul(
                                pk[k][64:128, :],
                                wss[k][64:128, j2, :],
                                rhs(dt_, PHI, (x0 + 4 + dx) * XPL + 16),
                                start=False, stop=last,
                                tile_position=(64, 64),
                            )

                    # epilogue: Winograd inverse along z (even = M0+M1+M2,
                    # odd = M1-M2-M3) via ACT copy (psum->sbuf) + DVE adds
                    # (single PSUM operand per op), then store.
                    osb = opool.tile([128, 512, 2], f32, tag="osb")
                    ev = osb[:, :, 0]
                    od = osb[:, :, 1]
                    import concourse.mybir as mybir

                    # column-halved so ACT and DVE pipeline (halves the
                    # exposed epilogue latency and PSUM-free delay)
                    for h0 in (0, 256):
                        hs = slice(h0, h0 + 256)
                        evh, odh = ev[:, hs], od[:, hs]
                        nc.scalar.copy(evh, p01[:, 0, hs])
                        nc.vector.tensor_add(
                            out=evh, in0=p01[:, 1, hs], in1=evh
                        )
                        nc.vector.tensor_add(
                            out=evh, in0=p23[:, 0, hs], in1=evh
                        )
                        nc.scalar.copy(odh, p01[:, 1, hs])
                        # odh -= M2, odh -= M3 (one PSUM operand per op)
                        nc.vector.scalar_tensor_tensor(
                            out=odh, in0=p23[:, 0, hs], scalar=-1.0, in1=odh,
                            op0=mybir.AluOpType.mult, op1=mybir.AluOpType.add,
                        )
                        nc.vector.scalar_tensor_tensor(
                            out=odh, in0=p23[:, 1, hs], scalar=-1.0, in1=odh,
                            op0=mybir.AluOpType.mult, op1=mybir.AluOpType.add,
                        )
                    lo = osb[0:64].rearrange(
                        "p (x y zt) r -> p x y (zt r)", x=4, y=16, zt=ZT
                    )
                    hi = osb[64:128].rearrange(
                        "p (x y zt) r -> p x y (zt r)", x=4, y=16, zt=ZT
                    )
                    # stores on the HWDGE (sync) ring only: SWDGE stores
                    # would hold up the gpsimd teardown drain ~2us.
                    nc.sync.dma_start(out_h[:, w, x0 : x0 + 2, :, :], lo[:, 0:2])
                    nc.sync.dma_start(out_h[:, w, x0 + 2 : x0 + 4, :, :], lo[:, 2:4])
                    nc.sync.dma_start(out_h[:, w, x0 + 4 : x0 + 6, :, :], hi[:, 0:2])
                    nc.sync.dma_start(out_h[:, w, x0 + 6 : x0 + 8, :, :], hi[:, 2:4])

    _split_multiwaits(nc)
    return nc


def _make_tile_context(nc):
    from concourse.tile import TileContext

    class TC(TileContext):
        # stock teardown is drain -> barrier -> sem-clear -> barrier; the
        # final barrier only orders engine-stream ends and costs ~2us.
        def _drain_and_barrier(self, tick_clock, wait_clock):
            from concourse.vector_clock import ScopedClock

            nc = self.nc
            drain_inst = nc.sync.drain()
            wait_clock.add_sem_waits(
                drain_inst.ins, ScopedClock({None: tick_clock.global_clock})
            )
            nc.all_engine_barrier()
            assert self.sems is not None
            popped = nc._tile_sem_poison_stack.pop()
            assert popped is self._sem_poison
            nc.clear_and_free_semaphores(list(self.sems.allocated().values()))

    return TC(nc)


def _split_multiwaits(nc, max_waits=1):
    """The walrus build here rejects any instruction carrying more than one
    sync-wait ("Too many sync wait commands").  Tile attaches one wait per
    outstanding producer.  Move excess waits onto same-engine NoOps inserted
    immediately before the instruction - semantically identical."""
    import concourse.mybir as mybir

    n_split = 0
    for fn in nc.m.functions:
        for blk in fn.blocks:
            out = []
            for inst in list(blk.instructions):
                si = inst.sync_info
                if si is not None and si.on_wait and len(si.on_wait) > max_waits:
                    waits = list(si.on_wait)
                    extra = waits[:-max_waits]
                    for k in range(0, len(extra), max_waits):
                        nop = mybir.InstNoOp(
                            name=f"{inst.name}.w{k}", ins=[], outs=[]
                        )
                        nop.engine = inst.engine
                        nop.sync_info = mybir.SyncInfo(
                            on_wait=extra[k : k + max_waits], on_update=[]
                        )
                        nc.register_instruction(nop)
                        out.append(nop)
                        n_split += 1
                    si.on_wait = waits[-max_waits:]
                out.append(inst)
            blk.instructions = out
    return n_split


# compute dtype: "float16" (fastest, rel err ~4e-4) or "float32r"
DTYPE = "float16"


def _get_nc():
    if "nc" not in _CACHE:
        import concourse.mybir as mybir

        _CACHE["nc"] = _build_nc(getattr(mybir.dt, DTYPE))
    return _CACHE["nc"]


def _np_dtype():
    if DTYPE == "float16":
        return np.float16
    return np.float32


def _shard_inputs(inputs):
    nd = _np_dtype()
    x = np.asarray(inputs["inputs"], dtype=np.float32)
    wk = np.asarray(inputs["kernel"], dtype=np.float32)
    k5 = wk.reshape(3, 3, 3, 3, CIN, COUT)  # [dw, dx, dy, dz, ci, co]
    # weight transform Gw_k over dz
    w0, w1, w2 = k5[:, :, :, 0], k5[:, :, :, 1], k5[:, :, :, 2]
    gw = [w0, (w0 + w1 + w2) * 0.5, (w0 - w1 + w2) * 0.5, w2]
    wps, wss = [], []
    for k in range(4):
        g = gw[k].reshape(9, 3, CIN, COUT)  # [(dw,dx), dy, ci, co]
        wp = np.concatenate(
            [g[:, 0].transpose(1, 0, 2), g[:, 1].transpose(1, 0, 2)], axis=0
        )
        w2h = g[:, 2].transpose(1, 0, 2)
        ws_ = np.concatenate([w2h, w2h], axis=0)
        wps.append(np.ascontiguousarray(wp.astype(nd)))
        wss.append(np.ascontiguousarray(ws_.astype(nd)))
    in_maps = []
    for c in range(8):
        b, wc = c // 4, c % 4
        w0c = 4 * wc
        sl = x[b, :, w0c : w0c + 6]             # [CIN, 6, 18, 18, 18] fp32
        ze = sl[..., 0::2]                      # z even: 0,2,..,16 (9)
        zo = sl[..., 1::2]                      # z odd: 1,3,..,17 (9)
        # m_k[ci, s, X, Y, zt], zt = 0..7: windows z = 2zt .. 2zt+3
        m = [
            ze[..., 0:8] - ze[..., 1:9],        # x0 - x2
            zo[..., 0:8] + ze[..., 1:9],        # x1 + x2
            ze[..., 1:9] - zo[..., 0:8],        # x2 - x1
            zo[..., 0:8] - zo[..., 1:9],        # x1 - x3
        ]
        feeds = {}
        for k in range(4):
            mk = m[k].reshape(CIN, NSLAB, DC).astype(nd)
            dk = np.zeros((128, NSLAB, DKC), dtype=nd)
            dk[0:CIN, :, 8:DKC] = mk            # lo rows: m_k[c-8]
            dk[CIN:, :, 0:DC] = mk              # hi rows: m_k[c]
            feeds[f"d{k}"] = dk
            feeds[f"wp{k}"] = wps[k]
            feeds[f"ws{k}"] = wss[k]
        in_maps.append(feeds)
    return in_maps


def _gather_outputs(results):
    out = np.empty((B, COUT, NW * 4, SO, SO, SO), dtype=np.float32)
    for c in range(8):
        b, wc = c // 4, c % 4
        w0 = 4 * wc
        out[b, :, w0 : w0 + 4] = results[c]["out"]
    return out


def kernel(**inputs):
    from concourse.bass_utils import run_bass_kernel_spmd

    res = run_bass_kernel_spmd(_get_nc(), _shard_inputs(inputs), list(range(8)))
    return _gather_outputs(res.results)
